# revision 9
# baseline (speedup 1.0000x reference)
"""Trainium2 Bass kernel for nn_Codec (exponential-lr SGD codec rollout).

Math: the reference scan is affine in x. With lr_t = LR0*GAMMA**t and
c_0 = 0, c_{t+1} = (1-lr_t)*c_t + lr_t, the per-step outputs are
  spike_t = 0.5*(c_t - 1) * x + 0.5
  y_t     = c_{t+1} * x
so each of the 2*T output slices is a scalar affine map of x. The kernel
is therefore pure output-bandwidth: load the x shard once per core, emit
2*T scaled copies.

Sharding: rows of x split evenly across 8 cores (fully data parallel).
"""

import sys

import numpy as np

sys.path.insert(0, "/opt/trn_rl_repo")

import concourse.bass as bass
import concourse.bacc as bacc
import concourse.mybir as mybir
from concourse import tile
from concourse.bass_utils import run_bass_kernel_spmd

LR0 = 0.15
GAMMA = 0.95
N_CORES = 8
ROWS, COLS = 2048, 2048
SHARD = ROWS // N_CORES  # 256 rows per core
P = 128  # SBUF partitions

last_exec_time_ns = None

_nc_cache: dict[int, bass.Bass] = {}


def _coeffs(T: int) -> tuple[np.ndarray, np.ndarray]:
    lrs = LR0 * GAMMA ** np.arange(T, dtype=np.float64)
    c = np.zeros(T + 1)
    for t in range(T):
        c[t + 1] = (1.0 - lrs[t]) * c[t] + lrs[t]
    a_spike = (0.5 * (c[:T] - 1.0)).astype(np.float32)  # spike_t = a*x + 0.5
    a_y = c[1:].astype(np.float32)  # y_t = a*x
    return a_spike, a_y


def _build(T: int, repeat: int = 1) -> bass.Bass:
    a_spike, a_y = _coeffs(T)
    f32 = mybir.dt.float32

    nc = bacc.Bacc("TRN2", target_bir_lowering=False)
    x = nc.dram_tensor("x", [SHARD, COLS], f32, kind="ExternalInput")
    out = nc.dram_tensor("out", [2, T, SHARD, COLS], f32, kind="ExternalOutput")

    n_row_tiles = SHARD // P  # 2
    with tile.TileContext(nc) as tc:
        with (
            tc.tile_pool(name="xin", bufs=1) as xpool,
            tc.tile_pool(name="obuf", bufs=20) as opool,
        ):
            xts = []
            for i in range(n_row_tiles):
                xt = xpool.tile([P, COLS], f32, tag=f"x{i}")
                nc.sync.dma_start(xt[:], x[i * P : (i + 1) * P, :])
                xts.append(xt)

            def body():
                k = 0
                for t in range(T):
                    for s, a, b in ((0, a_spike[t], 0.5), (1, a_y[t], 0.0)):
                        for i in range(n_row_tiles):
                            ot = opool.tile([P, COLS], f32, tag="o")
                            if k % 2 == 0:
                                nc.vector.tensor_scalar(
                                    ot[:], xts[i][:], float(a), float(b),
                                    mybir.AluOpType.mult, mybir.AluOpType.add,
                                )
                            else:
                                nc.scalar.activation(
                                    ot[:], xts[i][:],
                                    mybir.ActivationFunctionType.Copy,
                                    bias=float(b), scale=float(a),
                                )
                            nc.sync.dma_start(
                                out[s, t, i * P : (i + 1) * P, :], ot[:]
                            )
                            k += 1

            if repeat == 1:
                body()
            else:  # bench-only: amplify HW time so it rises above dispatch floor
                with tc.For_i(0, repeat):
                    body()
    nc.finalize()
    return nc


_runner_cache: dict[int, tuple] = {}


def _make_runner(T: int):
    """Same execution mechanism as bass_utils.run_bass_kernel_spmd under axon
    (bass2jax _bass_exec_p via shard_map over 8 cores), but with a
    single-transfer gather: the spmd helper uploads 512 MB of donated zeros
    and fetches the concat buffer once per core (8x 512 MB); here the zero
    output operands live on device across calls (no donation -- the kernel
    writes every output element) and the result comes back in one transfer."""
    import jax
    from jax.sharding import Mesh, NamedSharding, PartitionSpec
    from jax.experimental.shard_map import shard_map
    from concourse import bass2jax

    nc = _nc_cache.setdefault(T, _build(T))
    bass2jax.install_neuronx_cc_hook()
    partition_name = nc.partition_id_tensor.name if nc.partition_id_tensor else None
    in_names, out_names, out_avals = [], [], []
    for alloc in nc.m.functions[0].allocations:
        if not isinstance(alloc, mybir.MemoryLocationSet):
            continue
        name = alloc.memorylocations[0].name
        if alloc.kind == "ExternalInput":
            if name != partition_name:
                in_names.append(name)
        elif alloc.kind == "ExternalOutput":
            out_names.append(name)
            out_avals.append(
                jax.core.ShapedArray(tuple(alloc.tensor_shape), mybir.dt.np(alloc.dtype))
            )
    assert in_names == ["x"] and out_names == ["out"]
    all_in_names = in_names + out_names + ([partition_name] if partition_name else [])

    def _body(*args):
        operands = list(args)
        if partition_name is not None:
            operands.append(bass2jax.partition_id_tensor())
        return tuple(
            bass2jax._bass_exec_p.bind(
                *operands,
                out_avals=tuple(out_avals),
                in_names=tuple(all_in_names),
                out_names=tuple(out_names),
                lowering_input_output_aliases=(),
                sim_require_finite=True,
                sim_require_nnan=True,
                nc=nc,
            )
        )

    devices = jax.devices()[:N_CORES]
    mesh = Mesh(np.asarray(devices), ("core",))
    n_in = len(in_names) + len(out_names)
    f = jax.jit(
        shard_map(_body, mesh=mesh, in_specs=(PartitionSpec("core"),) * n_in,
                  out_specs=(PartitionSpec("core"),) * len(out_names),
                  check_rep=False),
        keep_unused=True,
    )
    sharding = NamedSharding(mesh, PartitionSpec("core"))
    zshape = (N_CORES * out_avals[0].shape[0], *out_avals[0].shape[1:])
    dev_zero = jax.device_put(np.zeros(zshape, np.float32), sharding)
    return f, sharding, dev_zero


def kernel(x: np.ndarray, T) -> np.ndarray:
    T = int(T)
    x = np.ascontiguousarray(np.asarray(x), dtype=np.float32)

    try:
        import jax

        if T not in _runner_cache:
            _runner_cache[T] = _make_runner(T)
        f, sharding, dev_zero = _runner_cache[T]
        dev_x = jax.device_put(x, sharding)  # row-sharded: 256 rows per core
        (out_dev,) = f(dev_x, dev_zero)
        flat = np.asarray(out_dev)  # one device->host transfer
        # [8*2, T, SHARD, COLS] -> [2, T, 8*SHARD, COLS]
        return np.ascontiguousarray(
            flat.reshape(N_CORES, 2, T, SHARD, COLS)
            .transpose(1, 2, 0, 3, 4)
            .reshape(2, T, ROWS, COLS)
        )
    except Exception:
        # proven-path fallback
        nc = _nc_cache.setdefault(T, _build(T))
        in_maps = [{"x": x[i * SHARD : (i + 1) * SHARD]} for i in range(N_CORES)]
        res = run_bass_kernel_spmd(nc, in_maps, list(range(N_CORES)))
        return np.concatenate([r["out"] for r in res.results], axis=2)
